# revision 13
# baseline (speedup 1.0000x reference)
"""Distributed exact k-NN retrieval (scores + top-k + gather) on 8 Trainium2
NeuronCores.

Strategy (standard distributed ANN): mat is sharded row-wise across the 8
cores. Each core streams its 64 MB shard once from HBM, computes the f32
scores mat_shard @ query on DVE+ACT (fused multiply-accumulate split across
both engines so neither exceeds the DMA roofline), then uses the DVE Max8 /
MaxIndex8 instructions to produce its per-partition top-8 candidates
(128*8 = 1024 per core) and gathers their rows with an indirect DMA. The
host merges the 8 * 1024 candidates down to the global top-k.

Device-side shard layout: rows are permuted host-side so partition p holds
rows p*496 .. p*496+495 of the (padded) shard, in order. Free-dim position
j of the score buffer then directly addresses shard row p*496 + j, so the
indirect-gather offsets need only one integer add (exact even in the DVE's
fp32 ALUs) instead of bit arithmetic.

Per-partition top-8 covers the global top-64 as long as no (core, partition)
bin holds more than 8 of the global top-64 rows. Rows are spread over 1024
bins; for 64 rows P[any bin >= 9] ~ 1e-14; verified against the actual
dataset in test.py (max observed bin load: 2).
"""
import numpy as np

import concourse.bacc as bacc
import concourse.bass as bass
import concourse.mybir as mybir
import concourse.tile as tile
from concourse.bass_utils import run_bass_kernel_spmd

P = 128                 # SBUF partitions
D = 256                 # feature dim
NCORES = 8
SHARD = 62500           # real rows per core
JCOLS = 496             # score columns = rows per partition (padded)
SHARD_PAD = P * JCOLS   # 63488 (zero-padded; pad scores are ~0, never top-8)
NCAND = 8               # candidates per partition (Max8)

# defaults (tuned)
CH = 16                 # rows per partition per DMA tile; must divide JCOLS
Y = 16                  # chunks per tile fused on DVE (all: ACT accum path unused)
MAT_BUFS = 6
ALT_RINGS = True
PROD_MAX = 12           # max chunks per bulk-product piece (SBUF pressure)

_F32 = mybir.dt.float32
_U32 = mybir.dt.uint32


def _build_module(repeat=1, mat_bufs=MAT_BUFS, y=Y, ch=CH, alt_rings=ALT_RINGS):
    assert JCOLS % ch == 0
    nt = JCOLS // ch
    nc = bacc.Bacc("TRN2", target_bir_lowering=False, debug=False,
                   enable_asserts=False)
    mat = nc.dram_tensor("mat", [SHARD_PAD, D], _F32, kind="ExternalInput").ap()
    qw = nc.dram_tensor("qw", [P, ch * D], _F32, kind="ExternalInput").ap()
    vals8 = nc.dram_tensor("vals8", [P, NCAND], _F32, kind="ExternalOutput").ap()
    idx8 = nc.dram_tensor("idx8", [P, NCAND], _U32, kind="ExternalOutput").ap()
    rowsg = nc.dram_tensor("rowsg", [P, NCAND * D], _F32, kind="ExternalOutput").ap()

    # device rows are permuted: dev row p*JCOLS + t*ch + c
    mat_r = mat.rearrange("(p t c) d -> t p (c d)", p=P, c=ch)

    with tile.TileContext(nc) as tc, \
            tc.tile_pool(name="const", bufs=1) as const_pool, \
            tc.tile_pool(name="mat", bufs=mat_bufs) as mat_pool, \
            tc.tile_pool(name="prod", bufs=3) as prod_pool, \
            tc.tile_pool(name="scr", bufs=3) as scr_pool, \
            tc.tile_pool(name="misc", bufs=1) as misc_pool:

        q_sb = const_pool.tile([P, ch * D], _F32)
        nc.sync.dma_start(q_sb[:], qw[:])

        pp = misc_pool.tile([P, NCAND], _U32, tag="pp")
        nc.gpsimd.iota(pp[:], pattern=[[0, NCAND]], base=0,
                       channel_multiplier=JCOLS)

        def body():
            scores = misc_pool.tile([P, JCOLS], _F32, tag="scores")

            for t in range(nt):
                mt = mat_pool.tile([P, ch * D], _F32, tag="mt")
                # alternate between the two HWDGE rings (SP / ACT) so the
                # per-DMA fixed completion cost overlaps across rings
                eng = nc.scalar if (alt_rings and t % 2) else nc.sync
                eng.dma_start(mt[:], mat_r[t])
                # fused multiply + free-dim sum on DVE for the first y chunks
                for c in range(y):
                    scr = scr_pool.tile([P, D], _F32, tag="fscr")
                    nc.vector.scalar_tensor_tensor(
                        out=scr[:],
                        in0=mt[:, c * D:(c + 1) * D],
                        scalar=0.0,
                        in1=q_sb[:, c * D:(c + 1) * D],
                        op0=mybir.AluOpType.bypass,
                        op1=mybir.AluOpType.mult,
                        accum_out=scores[:, t * ch + c: t * ch + c + 1],
                    )
                # bulk elementwise product on DVE (in pieces), accumulate
                # each chunk on ACT
                c0 = y
                while c0 < ch:
                    c1 = min(c0 + PROD_MAX, ch)
                    prod = prod_pool.tile([P, (c1 - c0) * D], _F32, tag="prod")
                    nc.vector.tensor_tensor(
                        prod[:], mt[:, c0 * D:c1 * D], q_sb[:, c0 * D:c1 * D],
                        mybir.AluOpType.mult,
                    )
                    for c in range(c0, c1):
                        ascr = scr_pool.tile([P, D], _F32, tag="ascr")
                        nc.scalar.activation(
                            out=ascr[:],
                            in_=prod[:, (c - c0) * D:(c - c0 + 1) * D],
                            func=mybir.ActivationFunctionType.Copy,
                            accum_out=scores[:, t * ch + c: t * ch + c + 1],
                        )
                    c0 = c1

            # per-partition top-8 (values descending) + free-dim positions
            v8 = misc_pool.tile([P, NCAND], _F32, tag="v8")
            i8 = misc_pool.tile([P, NCAND], _U32, tag="i8")
            nc.vector.max(out=v8[:], in_=scores[:])
            nc.vector.max_index(out=i8[:], in_max=v8[:], in_values=scores[:])
            nc.sync.dma_start(vals8[:], v8[:])
            nc.sync.dma_start(idx8[:], i8[:])

            # shard-row ids: r = p*JCOLS + j  (single exact integer add)
            r = misc_pool.tile([P, NCAND], _U32, tag="r")
            nc.vector.tensor_tensor(r[:], i8[:], pp[:], mybir.AluOpType.add)

            # gather the 1024 candidate rows from the DRAM shard. One call
            # per candidate rank: the HW DGE only honors per-partition
            # indirect offsets with a single offset per partition ([P, 1]).
            rows_sb = misc_pool.tile([P, NCAND * D], _F32, tag="rows")
            for jj in range(NCAND):
                nc.gpsimd.indirect_dma_start(
                    out=rows_sb[:, jj * D:(jj + 1) * D],
                    out_offset=None,
                    in_=mat[:],
                    in_offset=bass.IndirectOffsetOnAxis(ap=r[:, jj:jj + 1], axis=0),
                )
            nc.sync.dma_start(rowsg[:], rows_sb[:])

        if repeat == 1:
            body()
        else:
            with tc.For_i(0, repeat, 1):
                body()

    nc.compile()
    return nc


_NC_CACHE = {}


def _get_module(**kw):
    key = tuple(sorted(kw.items()))
    if key not in _NC_CACHE:
        _NC_CACHE[key] = _build_module(**kw)
    return _NC_CACHE[key]


def _prep_shards(mat, ch=CH):
    """Pad to SHARD_PAD rows/core and permute into the device layout:
    dev[core, p*JCOLS + t*ch + c] = orig[core, t*(P*ch) + p*ch + c]."""
    nt = JCOLS // ch
    padded = np.zeros((NCORES, SHARD_PAD, D), dtype=np.float32)
    padded[:, :SHARD] = np.asarray(mat, np.float32).reshape(NCORES, SHARD, D)
    dev = padded.reshape(NCORES, nt, P, ch, D).transpose(0, 2, 1, 3, 4)
    return np.ascontiguousarray(dev.reshape(NCORES, SHARD_PAD, D))


def _make_qw(query, ch=CH):
    return np.ascontiguousarray(np.tile(np.asarray(query, np.float32), (P, ch)))


def _run_device(shards, qw, trace=False, repeat=1, **kw):
    nc = _get_module(repeat=repeat, **kw)
    in_maps = [{"mat": shards[i], "qw": qw} for i in range(NCORES)]
    return run_bass_kernel_spmd(nc, in_maps, core_ids=list(range(NCORES)),
                                trace=trace)


def _merge(res, k, ch=CH):
    vals = np.stack([r["vals8"] for r in res.results])            # [8,128,8] f32
    idxs = np.stack([r["idx8"] for r in res.results])             # [8,128,8] u32
    rows = np.stack([r["rowsg"] for r in res.results])            # [8,128,8*D]
    rows = rows.reshape(NCORES, P, NCAND, D)

    j = idxs.astype(np.int64)                                     # j = t*ch + c
    p_term = (np.arange(P, dtype=np.int64) * ch)[None, :, None]
    r_orig = (j // ch) * (P * ch) + p_term + (j % ch)             # original row
    g = r_orig + (np.arange(NCORES, dtype=np.int64) * SHARD)[:, None, None]

    vals_f = vals.reshape(-1)
    g_f = g.reshape(-1)
    rows_f = rows.reshape(-1, D)
    valid = r_orig.reshape(-1) < SHARD
    vals_f, g_f, rows_f = vals_f[valid], g_f[valid], rows_f[valid]

    # jax.lax.top_k order: by value desc, ties -> lower index first
    sel = np.lexsort((g_f, -vals_f))[:k]
    idx = g_f[sel].astype(np.int32)
    out = np.ascontiguousarray(rows_f[sel], dtype=np.float32)
    return out, idx


def kernel(query, mat, k):
    k = int(k)
    shards = _prep_shards(mat)
    qw = _make_qw(query)
    res = _run_device(shards, qw)
    return _merge(res, k)
